# revision 6
# baseline (speedup 1.0000x reference)
"""Trainium2 Bass kernel for CRF negative-log-likelihood loss.

Problem: nn_CRF (B=512, L=1024, T=48), data-parallel over 8 NeuronCores
(64 batch rows per core). Each core computes a scalar partial loss; the
host sums the 8 partials.

Per-core algorithm (exp-domain forward scan + stream-based gold path):
  forward (partition function):
    bf16 scan A_t = (E^T A_{t-1}) o F_t with E = exp(trans - log T) bf16
    stationary, F_t = exp(featT - MU) bf16 produced by bulk bf16 PE
    transposes + fused ACT exp. Per-b renormalization every R steps is
    folded into the F tile DELTA steps later; the applied fp32
    reciprocals are stored and their Ln is taken once at the end
    (single ACT table swap). Terminal alphas: A_t for t >= CAPS go into
    an SBUF history; one bulk end-capture matmul per 8 steps produces
    exp(end)^T A_t rows which are staged to DRAM and Ln'd at the end
    under indicator selection (ind = maskT[t] - maskT[t+1]).
  gold (numerator): masked tag streams in (t,b) column order are staged
    through DRAM, partition-broadcast on GPSIMD, and consumed by fused
    scalar_tensor_tensor gathers: feat score directly against the
    transposed-feat PSUM tiles (no second feats read), transition score
    via a trans-row-select matmul (R = trans^T @ onehot) followed by a
    second stt against R. Start/end/last-step corrections as small
    b-partition ops.
"""

import math

import numpy as np

import concourse.bacc as bacc
import concourse.mybir as mybir
import concourse.tile as tile
from concourse.bass_utils import run_bass_kernel_spmd

F32 = mybir.dt.float32
BF16 = mybir.dt.bfloat16
I32 = mybir.dt.int32
AF = mybir.ActivationFunctionType
OP = mybir.AluOpType

B_FULL = 512
N_CORES = 8
BC = B_FULL // N_CORES  # 64
L_FULL = 1024
T = 48

MU = 0.51                # per-step feat shift folded into F
A_SHIFT = math.log(T)    # shift folded into E
R = 16                   # renorm period (steps)
DELTA = 8                # renorm application delay (steps)
FCHUNK = 32              # timesteps per F-prep DMA chunk
CAPS = 504               # first captured step (lengths >= L/2 = 512)


def build_program(L=L_FULL, Bc=BC):
    assert L % 128 == 0 and L % FCHUNK == 0
    n_tt = L // 128
    nchunks = L // FCHUNK
    ncap = L - CAPS                      # 520 captured steps
    ncapb = ncap // 8                    # 65 capture blocks
    n_cap_tiles = (L - 512) // 128       # 4 end Ln tiles (t=512..L-1)
    renorm_ts = [t for t in range(R, L + 1, R) if t + DELTA - 1 < L]
    nren = len(renorm_ts)                # 63

    nc = bacc.Bacc("TRN2", target_bir_lowering=False, debug=False)

    feats_d = nc.dram_tensor("feats", (Bc, L, T), F32, kind="ExternalInput")
    trans_d = nc.dram_tensor("trans", (T, T), F32, kind="ExternalInput")
    start_d = nc.dram_tensor("start", (T,), F32, kind="ExternalInput")
    end_d = nc.dram_tensor("end", (T,), F32, kind="ExternalInput")
    tags_d = nc.dram_tensor("tags", (Bc, L), I32, kind="ExternalInput")
    mask_d = nc.dram_tensor("mask", (Bc, L), I32, kind="ExternalInput")
    out_d = nc.dram_tensor("out", (1, 1), F32, kind="ExternalOutput")

    feats_flat = feats_d.ap().rearrange("b l t -> b (l t)")

    with tile.TileContext(nc) as tc:
        with (
            tc.tile_pool(name="const", bufs=1) as cp,
        ):
            # ---------------- constants ----------------
            intp_scope = tc.tile_pool(name="intp", bufs=1)
            intp = intp_scope.__enter__()
            iotaPi = intp.tile((T, 1), I32)
            nc.gpsimd.iota(iotaPi[:, :], [[1, 1]], channel_multiplier=1)
            iotaP = cp.tile((T, 1), F32)
            nc.vector.tensor_copy(iotaP[:, :], iotaPi[:, :])

            iota48i = intp.tile((Bc, T), I32)
            nc.gpsimd.iota(iota48i[:, :], [[1, T]], channel_multiplier=0)
            iota48f = cp.tile((Bc, T), F32)
            nc.vector.tensor_copy(iota48f[:, :], iota48i[:, :])

            iotaLi = intp.tile((Bc, L), I32)
            nc.gpsimd.iota(iotaLi[:, :], [[1, L]], channel_multiplier=0)
            iotaLf = intp.tile((Bc, L), F32)
            nc.vector.tensor_copy(iotaLf[:, :], iotaLi[:, :])

            iota64i = intp.tile((64, 64), I32)
            nc.gpsimd.iota(iota64i[:, :], [[1, 64]], channel_multiplier=0)
            iotaPi64 = intp.tile((64, 1), I32)
            nc.gpsimd.iota(iotaPi64[:, :], [[1, 1]], channel_multiplier=1)
            iota64f = intp.tile((64, 64), F32)
            nc.vector.tensor_copy(iota64f[:, :], iota64i[:, :])
            iotaPf64 = intp.tile((64, 1), F32)
            nc.vector.tensor_copy(iotaPf64[:, :], iotaPi64[:, :])
            identf = intp.tile((64, 64), F32)
            nc.vector.tensor_scalar(
                identf[:, :], iota64f[:, :], iotaPf64[:, :], None,
                OP.is_equal)
            identb = cp.tile((64, 64), BF16)
            nc.vector.tensor_copy(identb[:, :], identf[:, :])

            ones128 = cp.tile((128, 1), F32)
            nc.vector.memset(ones128[:, :], 1.0)
            ones48b = cp.tile((T, 1), BF16)
            nc.vector.memset(ones48b[:, :], 1.0)

            bias_a = cp.tile((T, 1), F32)
            nc.vector.memset(bias_a[:, :], -A_SHIFT)
            bias_mu = cp.tile((T, 1), F32)
            nc.vector.memset(bias_mu[:, :], -MU)

            # ---------------- params ----------------
            trans_sb = cp.tile((T, T), F32)
            nc.sync.dma_start(trans_sb[:, :], trans_d.ap())
            e_mat = cp.tile((T, T), BF16)
            nc.scalar.activation(e_mat[:, :], trans_sb[:, :], AF.Exp,
                                 bias=bias_a[:, :])
            transb = cp.tile((T, T), BF16)
            nc.gpsimd.tensor_copy(transb[:, :], trans_sb[:, :])

            end_sb = cp.tile((T, 1), F32)
            nc.sync.dma_start(end_sb[:, :], end_d.ap().unsqueeze(1))
            expend = cp.tile((T, 1), BF16)
            nc.scalar.activation(expend[:, :], end_sb[:, :], AF.Exp)

            start_sb = cp.tile((T, 1), F32)
            nc.sync.dma_start(start_sb[:, :], start_d.ap().unsqueeze(1))
            expstart = cp.tile((T, 1), F32)
            nc.scalar.activation(expstart[:, :], start_sb[:, :], AF.Exp)

            startbc = cp.tile((Bc, T), F32)
            nc.sync.dma_start(
                startbc[:, :],
                start_d.ap().unsqueeze(0).partition_broadcast(Bc))
            endbc = cp.tile((Bc, T), F32)
            nc.sync.dma_start(
                endbc[:, :],
                end_d.ap().unsqueeze(0).partition_broadcast(Bc))

            # ---------------- tags / mask ----------------
            tags_i = intp.tile((Bc, L), I32)
            nc.sync.dma_start(tags_i[:, :], tags_d.ap())
            tagsf = cp.tile((Bc, L), F32)
            nc.vector.tensor_copy(tagsf[:, :], tags_i[:, :])
            mask_i = intp.tile((Bc, L), I32)
            nc.sync.dma_start(mask_i[:, :], mask_d.ap())
            maskf = cp.tile((Bc, L), F32)
            nc.vector.tensor_copy(maskf[:, :], mask_i[:, :])

            # DRAM staging
            with tc.tile_pool(name="dramp", bufs=1, space="DRAM") as dp:
                stream_m_d = dp.tile((1, L * Bc), F32, name="stream_m")
                stream_s_d = dp.tile((1, L * Bc), F32, name="stream_s")
                cap_dram = dp.tile((1, ncap * Bc), F32, name="cap_stage")

            # persistent end-phase tiles
            ind = {}
            for k in range(3, n_tt):
                ind[k] = cp.tile((128, Bc), F32, name=f"ind_{k}")
            ind_c0 = cp.tile((1, Bc), F32)
            mrowbuf = cp.tile((1, nren * Bc), F32)
            lenm1_row = cp.tile((1, Bc), F32)
            rbuf = cp.tile((1, nren * Bc), F32)
            gacc_f = cp.tile((T, nchunks * 4), F32)
            gacc_t = cp.tile((T, nchunks * 4), F32)
            misc_acc = cp.tile((Bc, 4), F32)
            ahist = cp.tile((T, ncap * Bc), BF16)

            # ---------------- prep: transposed tag/mask ----------------
            with (
                tc.tile_pool(name="prep", bufs=2) as prp,
                tc.tile_pool(name="prepps", bufs=2, space="PSUM") as ppp,
            ):
                maskT = []
                tagsT = []
                for k in range(n_tt):
                    ps = ppp.tile((128, Bc), F32, name=f"tpm_{k}", tag="tp",
                                  bufs=2)
                    nc.tensor.transpose(
                        ps[:, :], maskf[:, 128 * k:128 * (k + 1)],
                        identf[:, :])
                    mt = prp.tile((128, Bc), F32, name=f"maskT_{k}",
                                  tag=f"mT{k}", bufs=1)
                    nc.vector.tensor_copy(mt[:, :], ps[:, :])
                    maskT.append(mt)
                    ps2 = ppp.tile((128, Bc), F32, name=f"tpt_{k}", tag="tp",
                                   bufs=2)
                    nc.tensor.transpose(
                        ps2[:, :], tagsf[:, 128 * k:128 * (k + 1)],
                        identf[:, :])
                    tt_ = prp.tile((128, Bc), F32, name=f"tagsT_{k}",
                                   tag=f"tT{k}", bufs=1)
                    nc.vector.tensor_copy(tt_[:, :], ps2[:, :])
                    tagsT.append(tt_)

                zero_row = cp.tile((1, Bc), F32)
                nc.vector.memset(zero_row[:, :], 0.0)

                for k in range(n_tt):
                    # shifted (t+1) mask/tags via partition-shift DMAs
                    ms = prp.tile((128, Bc), F32, name=f"maskTs_{k}",
                                  tag=f"ms{k}", bufs=1)
                    nc.sync.dma_start(ms[0:127, :], maskT[k][1:128, :])
                    ts_ = prp.tile((128, Bc), F32, name=f"tagsTs_{k}",
                                   tag=f"ts{k}", bufs=1)
                    nc.sync.dma_start(ts_[0:127, :], tagsT[k][1:128, :])
                    if k + 1 < n_tt:
                        nc.sync.dma_start(ms[127:128, :],
                                          maskT[k + 1][0:1, :])
                        nc.sync.dma_start(ts_[127:128, :],
                                          tagsT[k + 1][0:1, :])
                    else:
                        nc.sync.dma_start(ms[127:128, :], zero_row[:, :])
                        nc.sync.dma_start(ts_[127:128, :], zero_row[:, :])

                    # masked tags: tag + (1-mask)*100
                    off = prp.tile((128, Bc), F32, name="moff", tag="off",
                                   bufs=2)
                    nc.vector.tensor_scalar(off[:, :], maskT[k][:, :],
                                            -100.0, 100.0, OP.mult, OP.add)
                    tm = prp.tile((128, Bc), F32, name="tagsTm", tag="tm",
                                  bufs=2)
                    nc.vector.tensor_tensor(tm[:, :], tagsT[k][:, :],
                                            off[:, :], OP.add)
                    offs = prp.tile((128, Bc), F32, name="moffs", tag="off",
                                    bufs=2)
                    nc.vector.tensor_scalar(offs[:, :], ms[:, :],
                                            -100.0, 100.0, OP.mult, OP.add)
                    tms = prp.tile((128, Bc), F32, name="tagsTsm", tag="tm",
                                   bufs=2)
                    nc.vector.tensor_tensor(tms[:, :], ts_[:, :],
                                            offs[:, :], OP.add)

                    # fold masked tag tiles to DRAM streams ((t, b) order)
                    nc.sync.dma_start(
                        stream_m_d[0:1, k * 128 * Bc:(k + 1) * 128 * Bc]
                        .rearrange("o (p f) -> (o p) f", f=Bc),
                        tm[:, :])
                    nc.sync.dma_start(
                        stream_s_d[0:1, k * 128 * Bc:(k + 1) * 128 * Bc]
                        .rearrange("o (p f) -> (o p) f", f=Bc),
                        tms[:, :])

                    # indicator ind[t] = mask[t] - mask[t+1]
                    if k >= 3:
                        nc.vector.tensor_tensor(ind[k][:, :], maskT[k][:, :],
                                                ms[:, :], OP.subtract)

                nc.sync.dma_start(ind_c0[:, :], ind[3][127:128, :])

                # renorm mask rows (partition-0 packed)
                for r_i, t in enumerate(renorm_ts):
                    tf = t - 1 + DELTA
                    nc.sync.dma_start(
                        mrowbuf[0:1, r_i * Bc:(r_i + 1) * Bc],
                        maskT[tf // 128][tf % 128:tf % 128 + 1, :])

                # len row via ones-matmul over maskT
                len_ps = ppp.tile((1, Bc), F32, name="len_ps", tag="len",
                                  bufs=1)
                for k in range(n_tt):
                    nc.tensor.matmul(len_ps[:, :], ones128[:, :],
                                     maskT[k][:, :],
                                     start=(k == 0), stop=(k == n_tt - 1),
                                     skip_group_check=True)
                nc.vector.tensor_scalar(lenm1_row[:, :], len_ps[:, :], 1.0,
                                        None, OP.subtract)

            # ---------------- gold misc terms (b-partition) ----------------
            with (
                tc.tile_pool(name="miscp", bufs=2) as mp,
            ):
                featlast = mp.tile((Bc, T), F32, name="featlast", bufs=1)
                nc.sync.dma_start(featlast[:, :],
                                  feats_flat[:, (L - 1) * T:L * T])
                scrb = mp.tile((Bc, T), F32, name="scrb", tag="scrb")
                nc.vector.scalar_tensor_tensor(
                    scrb[:, :], iota48f[:, :], tagsf[:, 0:1],
                    startbc[:, :], OP.is_equal, OP.mult,
                    accum_out=misc_acc[:, 0:1])
                mtagl = mp.tile((Bc, 1), F32, name="mtagl", bufs=1)
                nc.vector.tensor_scalar(mtagl[:, :], maskf[:, L - 1:L],
                                        -100.0, 100.0, OP.mult, OP.add)
                nc.vector.tensor_tensor(mtagl[:, :], mtagl[:, :],
                                        tagsf[:, L - 1:L], OP.add)
                scrb2 = mp.tile((Bc, T), F32, name="scrb2", tag="scrb")
                fcor = mp.tile((Bc, 1), F32, name="fcor", bufs=1)
                nc.vector.scalar_tensor_tensor(
                    scrb2[:, :], iota48f[:, :], mtagl[:, :],
                    featlast[:, :], OP.is_equal, OP.mult,
                    accum_out=fcor[:, :])
                nc.vector.tensor_scalar(misc_acc[:, 3:4], fcor[:, :], -1.0,
                                        None, OP.mult)
                lenb = mp.tile((Bc, 1), F32, name="lenb", bufs=1)
                nc.vector.tensor_reduce(lenb[:, :], maskf[:, :],
                                        mybir.AxisListType.X, OP.add)
                lm1 = mp.tile((Bc, 1), F32, name="lm1", bufs=1)
                nc.vector.tensor_scalar(lm1[:, :], lenb[:, :], 1.0, None,
                                        OP.subtract)
                scrL = mp.tile((Bc, L), F32, name="scrL", bufs=1)
                lt = mp.tile((Bc, 1), F32, name="lt", bufs=1)
                nc.vector.scalar_tensor_tensor(
                    scrL[:, :], iotaLf[:, :], lm1[:, :], tagsf[:, :],
                    OP.is_equal, OP.mult, accum_out=lt[:, :])
                scrb3 = mp.tile((Bc, T), F32, name="scrb3", tag="scrb")
                nc.vector.scalar_tensor_tensor(
                    scrb3[:, :], iota48f[:, :], lt[:, :], endbc[:, :],
                    OP.is_equal, OP.mult, accum_out=misc_acc[:, 1:2])
                scrb4 = mp.tile((Bc, T), F32, name="scrb4", tag="scrb")
                fe0 = mp.tile((Bc, 1), F32, name="fe0", bufs=1)
                nc.vector.scalar_tensor_tensor(
                    scrb4[:, :], iota48f[:, :], lt[:, :], featlast[:, :],
                    OP.is_equal, OP.mult, accum_out=fe0[:, :])
                nc.vector.tensor_tensor(misc_acc[:, 2:3], fe0[:, :],
                                        maskf[:, L - 1:L], OP.mult)

            intp_scope.__exit__(None, None, None)

            # =============== scan + F-prep + gold streams ===============
            with (
                tc.tile_pool(name="natp", bufs=2) as natp,
                tc.tile_pool(name="natbp", bufs=2) as natbp,
                tc.tile_pool(name="stgp", bufs=2) as stgp,
                tc.tile_pool(name="fpool", bufs=10) as fpool,
                tc.tile_pool(name="bcp", bufs=2) as bcp,
                tc.tile_pool(name="ohp", bufs=3) as ohp,
                tc.tile_pool(name="scrp", bufs=2) as scrp,
                tc.tile_pool(name="srowp", bufs=2) as srowp,
                tc.tile_pool(name="apool", bufs=4) as apool,
                tc.tile_pool(name="tpps", bufs=3, space="PSUM") as tpps,
                tc.tile_pool(name="rpsp", bufs=2, space="PSUM") as rpsp,
                tc.tile_pool(name="scanps", bufs=1, space="PSUM") as scanps,
                tc.tile_pool(name="capps", bufs=1, space="PSUM") as capps,
                tc.tile_pool(name="csps", bufs=1, space="PSUM") as csps,
            ):
                ftiles = {}

                def emit_fprep(c):
                    natf = natp.tile((Bc, FCHUNK * T), F32, name="natf")
                    nc.sync.dma_start(
                        natf[:, :],
                        feats_flat[:, FCHUNK * T * c:FCHUNK * T * (c + 1)])
                    natb = natbp.tile((Bc, FCHUNK * T), BF16, name="natb")
                    nc.gpsimd.tensor_copy(natb[:, :], natf[:, :])
                    stgm = stgp.tile((1, FCHUNK * Bc), F32, name="stgm",
                                     tag="stgm")
                    nc.sync.dma_start(
                        stgm[0:1, :],
                        stream_m_d[0:1, FCHUNK * Bc * c:FCHUNK * Bc * (c + 1)])
                    stgs = stgp.tile((1, FCHUNK * Bc), F32, name="stgs",
                                     tag="stgs")
                    nc.sync.dma_start(
                        stgs[0:1, :],
                        stream_s_d[0:1, FCHUNK * Bc * c:FCHUNK * Bc * (c + 1)])
                    for q4 in range(FCHUNK // 8):
                        q = c * (FCHUNK // 8) + q4
                        ftp = tpps.tile((T, 8 * Bc), BF16, name="ftp")
                        for k in range(8):
                            blk = q4 * 8 + k
                            nc.tensor.transpose(
                                ftp[:, Bc * k:Bc * (k + 1)],
                                natb[:, T * blk:T * (blk + 1)],
                                identb[:, :])
                        ft = fpool.tile((T, 8 * Bc), BF16, name="ftile")
                        nc.scalar.activation(ft[:, :], ftp[:, :], AF.Exp,
                                             bias=bias_mu[:, :])
                        ftiles[q] = ft

                        # ---- gold for this 8-step block ----
                        cs0 = q4 * 8 * Bc
                        tagbc_m = bcp.tile((T, 8 * Bc), F32, name="tagbc_m",
                                           tag="bcm")
                        nc.gpsimd.partition_broadcast(
                            tagbc_m[:, :], stgm[0:1, cs0:cs0 + 8 * Bc],
                            channels=T)
                        tagbc_s = bcp.tile((T, 8 * Bc), F32, name="tagbc_s",
                                           tag="bcs")
                        nc.gpsimd.partition_broadcast(
                            tagbc_s[:, :], stgs[0:1, cs0:cs0 + 8 * Bc],
                            channels=T)
                        scrf = scrp.tile((T, 8 * Bc), F32, name="scrf",
                                         tag="scr")
                        nc.vector.scalar_tensor_tensor(
                            scrf[:, :], tagbc_m[:, :], iotaP[:, :],
                            ftp[:, :], OP.is_equal, OP.mult,
                            accum_out=gacc_f[:, q:q + 1])
                        ohuT = ohp.tile((T, 8 * Bc), BF16, name="ohuT")
                        nc.vector.tensor_scalar(
                            ohuT[:, :], tagbc_m[:, :], iotaP[:, :], None,
                            OP.is_equal)
                        rps = rpsp.tile((T, 8 * Bc), F32, name="rps")
                        nc.tensor.matmul(rps[:, :], transb[:, :],
                                         ohuT[:, :], start=True, stop=True,
                                         skip_group_check=True)
                        scrt = scrp.tile((T, 8 * Bc), F32, name="scrt",
                                         tag="scr")
                        nc.vector.scalar_tensor_tensor(
                            scrt[:, :], tagbc_s[:, :], iotaP[:, :],
                            rps[:, :], OP.is_equal, OP.mult,
                            accum_out=gacc_t[:, q:q + 1])

                def f_slice(t):
                    return ftiles[t // 8][0:T, (t % 8) * Bc:(t % 8 + 1) * Bc]

                emit_fprep(0)
                emit_fprep(1)

                # A0 = exp(start) * F_0
                a_prev = apool.tile((T, Bc), BF16, name="a_t")
                nc.vector.tensor_scalar(
                    a_prev[:, :], f_slice(0), expstart[:, :], None, OP.mult)

                renorm_set = set(renorm_ts)
                for t in range(1, L + 1):
                    if t % FCHUNK == 1:
                        c = (t - 1) // FCHUNK + 2
                        if c < nchunks:
                            emit_fprep(c)
                    tprev = t - 1
                    # bulk end-capture once a full 8-step history block done
                    if tprev >= CAPS + 7 and (tprev - (CAPS + 7)) % 8 == 0:
                        qb = (tprev - (CAPS + 7)) // 8
                        cap_ps = capps.tile((1, 8 * Bc), F32, name="cap_ps")
                        nc.tensor.matmul(
                            cap_ps[0:1, :], expend[:, :],
                            ahist[:, qb * 8 * Bc:(qb + 1) * 8 * Bc],
                            start=True, stop=True, skip_group_check=True)
                        crow = srowp.tile((1, 8 * Bc), F32, name="crow",
                                          tag="crow")
                        nc.vector.tensor_copy(crow[0:1, :], cap_ps[0:1, :])
                        nc.sync.dma_start(
                            cap_dram[0:1, qb * 8 * Bc:(qb + 1) * 8 * Bc],
                            crow[0:1, :])
                    # renorm: colsum of A_{t-1}; fold recip at t-1+DELTA
                    if t in renorm_set:
                        r_i = renorm_ts.index(t)
                        tf = t - 1 + DELTA
                        cs = csps.tile((1, Bc), F32, name="cs")
                        nc.tensor.matmul(
                            cs[:, :], ones48b[:, :], a_prev[:, :],
                            start=True, stop=True, skip_group_check=True)
                        nc.vector.reciprocal(
                            rbuf[0:1, r_i * Bc:(r_i + 1) * Bc], cs[:, :])
                        rbc = bcp.tile((T, Bc), F32, name="rbc", tag="rbc")
                        nc.gpsimd.partition_broadcast(
                            rbc[:, :], rbuf[0:1, r_i * Bc:(r_i + 1) * Bc],
                            channels=T)
                        nc.vector.tensor_tensor(
                            f_slice(tf), f_slice(tf), rbc[:, :], OP.mult)
                    if t < L:
                        ps = scanps.tile((T, Bc), F32, name="mm_ps")
                        nc.tensor.matmul(
                            ps[:, :], e_mat[:, :], a_prev[:, :],
                            start=True, stop=True, skip_group_check=True)
                        if t >= CAPS:
                            a_cur = ahist[0:T, (t - CAPS) * Bc:
                                          (t - CAPS + 1) * Bc]
                        else:
                            a_cur = apool.tile((T, Bc), BF16, name="a_t")
                            a_cur = a_cur[:, :]
                        nc.vector.tensor_tensor(a_cur, ps[:, :], f_slice(t),
                                                OP.mult)
                        a_prev = a_cur

                # final capture block (tprev = L-1)
                qb = ncapb - 1
                cap_ps = capps.tile((1, 8 * Bc), F32, name="cap_ps")
                nc.tensor.matmul(
                    cap_ps[0:1, :], expend[:, :],
                    ahist[:, qb * 8 * Bc:(qb + 1) * 8 * Bc],
                    start=True, stop=True, skip_group_check=True)
                crow = srowp.tile((1, 8 * Bc), F32, name="crow", tag="crow")
                nc.vector.tensor_copy(crow[0:1, :], cap_ps[0:1, :])
                nc.sync.dma_start(
                    cap_dram[0:1, qb * 8 * Bc:(qb + 1) * 8 * Bc],
                    crow[0:1, :])

            # =============== end phase ===============
            with (
                tc.tile_pool(name="endp", bufs=2) as ep,
                tc.tile_pool(name="endps", bufs=1, space="PSUM") as epp,
            ):
                # gold total
                gold_ps = epp.tile((1, 1), F32, name="gold_ps")
                gf = ep.tile((T, 1), F32, name="gf", bufs=1)
                nc.vector.tensor_reduce(gf[:, :], gacc_f[:, :],
                                        mybir.AxisListType.X, OP.add)
                gt = ep.tile((T, 1), F32, name="gt", bufs=1)
                nc.vector.tensor_reduce(gt[:, :], gacc_t[:, :],
                                        mybir.AxisListType.X, OP.add)
                gsum = ep.tile((T, 1), F32, name="gsum", bufs=1)
                nc.vector.tensor_tensor(gsum[:, :], gf[:, :], gt[:, :],
                                        OP.add)
                nc.tensor.matmul(gold_ps[:, :], ones128[0:T, :], gsum[:, :],
                                 start=True, stop=False,
                                 skip_group_check=True)
                mred = ep.tile((Bc, 1), F32, name="mred", bufs=1)
                nc.vector.tensor_reduce(mred[:, :], misc_acc[:, :],
                                        mybir.AxisListType.X, OP.add)
                nc.tensor.matmul(gold_ps[:, :], ones128[0:Bc, :],
                                 mred[:, :], start=False, stop=True,
                                 skip_group_check=True)

                # deferred renorm log-accounting:
                # logsel[b] = sum_r -ln(recip_r[b]) * mrow_r[b]
                lnbuf = ep.tile((1, nren * Bc), F32, name="lnbuf", bufs=1)
                nc.scalar.activation(lnbuf[0:1, :], rbuf[0:1, :], AF.Ln)
                nc.vector.tensor_tensor(lnbuf[0:1, :], lnbuf[0:1, :],
                                        mrowbuf[0:1, :], OP.mult)
                logsel = ep.tile((1, Bc), F32, name="logsel", bufs=1)
                nc.vector.tensor_reduce(
                    logsel[0:1, :],
                    lnbuf[0:1, :].rearrange("o (r b) -> o b r", b=Bc),
                    mybir.AxisListType.X, OP.add)

                # fwd from captures
                fwd_ps = epp.tile((1, Bc), F32, name="fwd_ps")
                for m in range(n_cap_tiles):
                    capt = ep.tile((128, Bc), F32, name="capt", tag="capt")
                    nc.sync.dma_start(
                        capt[:, :],
                        cap_dram[0:1, (8 + 128 * m) * Bc:
                                 (8 + 128 * (m + 1)) * Bc]
                        .rearrange("o (p f) -> (o p) f", f=Bc))
                    lc = ep.tile((128, Bc), F32, name="lc", tag="lc")
                    nc.scalar.activation(lc[:, :], capt[:, :], AF.Ln)
                    pr = ep.tile((128, Bc), F32, name="pr", tag="pr")
                    nc.vector.tensor_tensor(pr[:, :], lc[:, :],
                                            ind[4 + m][:, :], OP.mult)
                    nc.tensor.matmul(fwd_ps[:, :], ones128[:, :], pr[:, :],
                                     start=(m == 0),
                                     stop=(m == n_cap_tiles - 1),
                                     skip_group_check=True)
                fwd_sel = ep.tile((1, Bc), F32, name="fwd_sel", bufs=1)
                nc.vector.tensor_copy(fwd_sel[:, :], fwd_ps[:, :])
                cap0t = ep.tile((1, Bc), F32, name="cap0t", bufs=1)
                nc.sync.dma_start(cap0t[:, :], cap_dram[0:1, 7 * Bc:8 * Bc])
                lc0 = ep.tile((1, Bc), F32, name="lc0", bufs=1)
                nc.scalar.activation(lc0[:, :], cap0t[:, :], AF.Ln)
                nc.vector.tensor_tensor(lc0[:, :], lc0[:, :], ind_c0[:, :],
                                        OP.mult)
                nc.vector.tensor_tensor(fwd_sel[:, :], fwd_sel[:, :],
                                        lc0[:, :], OP.add)
                # + logsel (ln(cs) = -ln(recip)) and per-step shifts
                nc.vector.tensor_tensor(fwd_sel[:, :], fwd_sel[:, :],
                                        logsel[:, :], OP.subtract)
                shifts = ep.tile((1, Bc), F32, name="shifts", bufs=1)
                nc.vector.tensor_scalar(shifts[:, :], lenm1_row[:, :],
                                        A_SHIFT + MU, MU, OP.mult, OP.add)
                nc.vector.tensor_tensor(fwd_sel[:, :], fwd_sel[:, :],
                                        shifts[:, :], OP.add)
                fwd_tot = ep.tile((1, 1), F32, name="fwd_tot", bufs=1)
                nc.vector.tensor_reduce(fwd_tot[:, :], fwd_sel[:, :],
                                        mybir.AxisListType.X, OP.add)
                loss = ep.tile((1, 1), F32, name="loss", bufs=1)
                nc.vector.tensor_tensor(loss[:, :], fwd_tot[:, :],
                                        gold_ps[:, :], OP.subtract)
                nc.sync.dma_start(out_d.ap(), loss[:, :])

    nc.compile()
    return nc


def shard_inputs(feats, transitions, start_transitions, end_transitions,
                 tags, mask, n_cores=N_CORES):
    feats = np.ascontiguousarray(np.asarray(feats, dtype=np.float32))
    transitions = np.ascontiguousarray(
        np.asarray(transitions, dtype=np.float32))
    start_transitions = np.ascontiguousarray(
        np.asarray(start_transitions, dtype=np.float32))
    end_transitions = np.ascontiguousarray(
        np.asarray(end_transitions, dtype=np.float32))
    tags = np.ascontiguousarray(np.asarray(tags).astype(np.int32))
    mask = np.ascontiguousarray(np.asarray(mask).astype(np.int32))
    Bc = feats.shape[0] // n_cores
    in_maps = []
    for c in range(n_cores):
        s = slice(c * Bc, (c + 1) * Bc)
        in_maps.append({
            "feats": feats[s],
            "trans": transitions,
            "start": start_transitions,
            "end": end_transitions,
            "tags": tags[s],
            "mask": mask[s],
        })
    return in_maps, feats.shape


def kernel(feats, transitions, start_transitions, end_transitions, tags,
           mask, **_ignored):
    in_maps, (Bf, L, _) = shard_inputs(
        feats, transitions, start_transitions, end_transitions, tags, mask)
    nc = build_program(L=L, Bc=Bf // N_CORES)
    res = run_bass_kernel_spmd(nc, in_maps, core_ids=list(range(N_CORES)))
    total = sum(float(r["out"][0, 0]) for r in res.results)
    return np.float32(total)
